# revision 12
# baseline (speedup 1.0000x reference)
"""Trainium2 Bass kernel: 3x3 valid conv (64ch -> 128ch) + per-pixel bias.

Strategy: shard the 510 output rows spatially across 8 NeuronCores (64
rows/core with a 2-row input halo; core 7 overlaps core 6 by 2 rows).
Inside a core, the 64-row band is split across the two PE row-strips:
partitions 0-63 hold the input rows for output rows 0-31 of the band,
partitions 64-127 the rows for output rows 32-63 (the host feeds the
band pre-split so every DMA runs at full 128-partition width).  Each
output row is 9 accumulating K=64 fp16 matmuls (one per kernel tap,
N=510); the two strips run concurrently on the PE halves, so a
tap-pair costs one N=510 stream (~233 ns) and the matmul stream is the
hard floor (~67 us).  Bias is added during PSUM evacuation on the
Vector engine.

Everything rides as fp16 (x, w, bias, y; PSUM accumulates fp32): the
total HBM traffic drops to ~21 MB/core, well under the ~390 GB/s
fabric for the length of the matmul stream, so no DMA stall can reach
the PE.  The head is trimmed by loading the weights and the first
input chunk on three separate DMA rings (gpsimd / sync / vector) the
moment the queues go live, while ~36 tiny dummy matmuls on a zeroed
scratch tile ramp the PE out of its low power state so the real
stream starts at full clock.  The tail is trimmed with per-row stores
of the last group spread across idle rings.  fp16 keeps 10 mantissa
bits; pipeline error vs the fp32 reference is ~5e-4.
"""

import numpy as np
from contextlib import ExitStack

import concourse.bass as bass
import concourse.tile as tile
from concourse import bacc, mybir
from concourse import bass_utils

C, H, W = 64, 512, 512
D, KK = 128, 3
OH, OW = H - KK + 1, W - KK + 1          # 510, 510
NCORES = 8
RPC = 64                                  # output rows per core
BAND = RPC + KK - 1                       # 66 input rows per core
HALF = RPC // 2                           # 32 output rows per strip
IBAND = HALF + KK - 1                     # 34 input rows per strip
GROUPS = 8
GROWS = HALF // GROUPS                    # 4 pair-rows per group

f32 = mybir.dt.float32
f16 = mybir.dt.float16

# row offset of each core's output band
STARTS = [min(i * RPC, OH - RPC) for i in range(NCORES)]

_CACHE = {}

# results of the last hardware run (inspected by test harnesses)
LAST_RESULTS = None


def _build_program():
    nc = bacc.Bacc(
        "TRN2", target_bir_lowering=False, debug=False, num_devices=NCORES
    )
    # x is pre-split on the host: row (h*64+c) holds band rows
    # [32h, 32h+34) of channel c, flattened
    x = nc.dram_tensor("x", [2 * C, IBAND * W], f16, kind="ExternalInput").ap()
    # w is pre-duplicated: rows 0-63 and 64-127 identical, [c, (ky kx d)]
    w = nc.dram_tensor("w", [2 * C, 9 * D], f16, kind="ExternalInput").ap()
    b = nc.dram_tensor("b", [D, RPC, OW], f16, kind="ExternalInput").ap()
    y = nc.dram_tensor("y", [D, RPC, OW], f16, kind="ExternalOutput").ap()

    b_flat = b.rearrange("d r x -> d (r x)")
    y_flat = y.rearrange("d r x -> d (r x)")

    with tile.TileContext(nc) as tc:
        with ExitStack() as ctx:
            xp = ctx.enter_context(tc.tile_pool(name="xin", bufs=1))
            wp = ctx.enter_context(tc.tile_pool(name="wt", bufs=1))
            bp = ctx.enter_context(tc.tile_pool(name="bias", bufs=3))
            op = ctx.enter_context(tc.tile_pool(name="out", bufs=3))
            pp = ctx.enter_context(tc.tile_pool(name="ps", bufs=4, space="PSUM"))
            sp = ctx.enter_context(tc.tile_pool(name="scr", bufs=1))

            wt = wp.tile([128, 9 * D], f16)
            xin = xp.tile([128, IBAND * W], f16)

            # head: tap-0 weights land first so LDWEIGHTS can pre-issue;
            # the first chunk (3 rows — exactly output row 0's taps) splits
            # its strips across the two HWDGE rings, the remaining weights
            # split likewise right behind, then the input chunks and the
            # first bias tiles interleave on the scalar ring
            nc.sync.dma_start(wt[:, 0:D], w[:, 0:D])
            nc.sync.dma_start(xin[0:64, 0:3 * W], x[0:64, 0:3 * W])
            nc.scalar.dma_start(xin[64:128, 0:3 * W], x[64:128, 0:3 * W])
            nc.sync.dma_start(wt[0:64, D:9 * D], w[0:64, D:9 * D])
            nc.scalar.dma_start(wt[64:128, D:9 * D], w[64:128, D:9 * D])

            def load_bias(g, eng=None):
                eng = eng or nc.sync
                ra, rb = g * GROWS, HALF + g * GROWS
                ba = bp.tile([128, GROWS * OW], f16, tag="ba")
                eng.dma_start(ba[:], b_flat[:, ra * OW:(ra + GROWS) * OW])
                bb = bp.tile([128, GROWS * OW], f16, tag="bb")
                eng.dma_start(bb[:], b_flat[:, rb * OW:(rb + GROWS) * OW])
                return ba, bb

            # remaining chunks on the scalar ring, group-0 bias interleaved
            # so row 0's evac isn't gated on the whole bias batch
            bias_tiles = {}
            for ci, (r0, r1) in enumerate(
                [(3, 6), (6, 10), (10, 16), (16, 24), (24, IBAND)]
            ):
                nc.scalar.dma_start(xin[:, r0 * W:r1 * W], x[:, r0 * W:r1 * W])
                if ci == 0:
                    bias_tiles[0] = load_bias(0, nc.scalar)
            bias_tiles[1] = load_bias(1, nc.sync)

            # PE warm-up: short dummy matmuls on a zeroed scratch tile keep
            # the array busy from the end of the preamble so it reaches its
            # full power state before the real weights/input arrive
            warm = sp.tile([128, 256], f16)
            nc.vector.memset(warm[:], 0.0)
            pwarm = pp.tile([128, OW], f32, tag="pa")
            for _ in range(36):
                nc.tensor.matmul(
                    pwarm[:, 0:128],
                    warm[0:64, 0:128],
                    warm[0:64, 128:256],
                    start=True, stop=True,
                )

            for g in range(GROUPS):
                ra = g * GROWS                 # band rows ra..ra+3  (strip 0)
                rb = HALF + ra                 # band rows rb..rb+3  (strip 1)
                if g + 2 < GROUPS and g + 2 not in bias_tiles:
                    bias_tiles[g + 2] = load_bias(g + 2)
                ba, bb = bias_tiles.pop(g)
                ya = op.tile([128, GROWS * OW], f16, tag="ya")
                yb = op.tile([128, GROWS * OW], f16, tag="yb")

                for j in range(GROWS):
                    yl = ra + j                # strip-local output row
                    pa = pp.tile([128, OW], f32, tag="pa")
                    pb = pp.tile([128, OW], f32, tag="pb")
                    for t in range(9):
                        ky, kx = divmod(t, 3)
                        off = (yl + ky) * W + kx
                        nc.tensor.matmul(
                            pa[:],
                            wt[0:64, t * D:(t + 1) * D],
                            xin[0:64, off:off + OW],
                            start=(t == 0), stop=(t == 8),
                        )
                        nc.tensor.matmul(
                            pb[:],
                            wt[64:128, t * D:(t + 1) * D],
                            xin[64:128, off:off + OW],
                            start=(t == 0), stop=(t == 8),
                        )
                    sl = slice(j * OW, (j + 1) * OW)
                    nc.vector.tensor_add(ya[:, sl], pa[:], ba[:, sl])
                    nc.vector.tensor_add(yb[:, sl], pb[:], bb[:, sl])
                    if g == GROUPS - 1:
                        # tail: per-row stores on both HW rings so the
                        # final drain starts as soon as each row lands
                        nc.scalar.dma_start(
                            y_flat[:, (ra + j) * OW:(ra + j + 1) * OW],
                            ya[:, sl],
                        )
                        nc.sync.dma_start(
                            y_flat[:, (rb + j) * OW:(rb + j + 1) * OW],
                            yb[:, sl],
                        )

                if g < 5:
                    # early groups: scalar ring (its input chunks are done)
                    nc.scalar.dma_start(
                        y_flat[:, ra * OW:(ra + GROWS) * OW], ya[:]
                    )
                    nc.scalar.dma_start(
                        y_flat[:, rb * OW:(rb + GROWS) * OW], yb[:]
                    )
                elif g < GROUPS - 1:
                    # sync ring is past the bias stream by now
                    nc.sync.dma_start(
                        y_flat[:, ra * OW:(ra + GROWS) * OW], ya[:]
                    )
                    nc.sync.dma_start(
                        y_flat[:, rb * OW:(rb + GROWS) * OW], yb[:]
                    )

    nc.compile()
    return nc


def kernel(input, kernels, biases):
    global LAST_RESULTS
    if "nc" not in _CACHE:
        _CACHE["nc"] = _build_program()
    nc = _CACHE["nc"]

    xh = np.ascontiguousarray(input).astype(np.float16)        # [C, H, W]
    w1 = np.ascontiguousarray(
        kernels.transpose(1, 2, 3, 0)
    ).reshape(C, 9 * D).astype(np.float16)
    wr = np.concatenate([w1, w1], axis=0)                      # [128, 9*D]
    bh = np.ascontiguousarray(biases).astype(np.float16)

    in_maps = []
    for s in STARTS:
        band = xh[:, s:s + BAND, :]
        xs = np.concatenate(
            [band[:, 0:IBAND, :], band[:, HALF:HALF + IBAND, :]], axis=0
        ).reshape(2 * C, IBAND * W)
        in_maps.append({
            "x": np.ascontiguousarray(xs),
            "w": wr,
            "b": np.ascontiguousarray(bh[:, s:s + RPC, :]),
        })

    res = bass_utils.run_bass_kernel_spmd(
        nc, in_maps, core_ids=list(range(NCORES))
    )
    LAST_RESULTS = res

    out = np.empty((D, OH, OW), np.float32)
    for i, s in enumerate(STARTS):
        out[:, s:s + RPC, :] = res.results[i]["y"].astype(np.float32)
    return out
